# revision 10
# baseline (speedup 1.0000x reference)
"""Trainium2 Bass kernel for CropConv: 3x3 same-padding conv (64->64 ch) on
[16, 64, 128, 128] fp32 input, with a static crop mask zeroing output rows/cols
[44:84).

Strategy (data-parallel over batch, 8 cores x 2 images each):
  - Host marshals x into a zero-padded row-major layout with row stride 129
    (131 padded rows; the left zero column of each row doubles as the previous
    row's right pad), so every conv tap (kh, kw) of an output row-chunk is one
    contiguous rhs slice.
  - Per core, image 0 lives in SBUF partitions 0-63 (partition = in-channel),
    image 1 in partitions 64-127.
  - The conv is 9 PSUM-accumulated TensorE matmuls per output chunk:
    out[oc, pix] += W[kh,kw][ic, oc].T @ x[ic, shifted pix].  K = M = 64, so
    four matmuls run concurrently in the four 64x64 quadrants of the PE array
    (row-half = image, col-half = chunk parity within the pair (2p, 2p+1)).
  - One pair (2 chunks x 2 images) per PSUM-bank group: finest-grained x-row
    dependency so the matmul stream starts as soon as the first 8 padded rows
    land and never starves (a starved >1us gap costs double: the HAM clock
    gate re-throttles the PE to 50% for its next ~3.4us window).
  - x input segments are interleaved across the sync and scalar HWDGE rings
    in strictly increasing row order so rows always arrive ahead of compute.
  - The crop mask is applied with cheap GPSIMD memsets on the staged output
    (no mask tensor, no mask DMA) and evictions are uniform: PSUM -> fp16
    stage via Vector (img 0) and Scalar-ACT (img 1) copies.
  - Stores are batched 4 slots early on; the leftover chunk 42 (rows 126/127,
    computed mid-stream at full four-quadrant width) and the last two pairs
    are stored individually so the post-matmul tail is short.
  - PE warm-up: full-width dummy matmuls during the initial x-load wait keep
    the HAM util window high so full clock is granted by the time real data
    lands.  The host untangles the chunk-major fp16 output and upcasts.
"""

import numpy as np

# ---- problem constants (hardcoded; kernel.py must be self-contained) ----
B, C, H, W = 16, 64, 128, 128
OC, KS = 64, 3
N_CORES = 8
IMGS = B // N_CORES  # 2 images per core

WP = W + 1            # padded row stride: 129
HP = H + 3            # padded rows in the x buffer: 131
XLEN = HP * WP        # 16899 elems per partition

RPC = 3               # output rows per chunk
NCH = (H + RPC - 1) // RPC   # 43 chunks per image (last has 2 rows)
NCHP = NCH + 1        # padded to 44 (chunk 43 is never written, dropped)
NPAIR = 21            # adjacent-chunk pairs (2p, 2p+1); chunk 42 leftover
NSLOT = 22            # stage slots per image: 21 pairs + leftover
CHN = RPC * WP        # matmul free dim per full chunk: 387
CHS = RPC * W         # compact stage slot stride: 384
STLEN = 2 * NSLOT * CHS   # stage free size: 16896

CROP0, CROP1 = 44, 84  # masked rows/cols [44, 84)
# PE warm-up matmuls: must bridge the whole preamble-end .. first-x-rows-land
# window with NO gap, or the HAM utilization window (~3.4us granularity,
# ~75% threshold) misses the grant and the PE runs at 50% for extra windows.
N_DUMMY = 32

_CACHE = {}


def _build_module():
    import concourse.tile as tile
    from concourse import bacc, mybir

    f32 = mybir.dt.float32
    f16 = mybir.dt.float16
    bf16 = mybir.dt.bfloat16

    nc = bacc.Bacc("TRN2", target_bir_lowering=False, debug=False,
                   num_devices=N_CORES)

    x_ap = nc.dram_tensor("xin", [IMGS, C, XLEN], bf16,
                          kind="ExternalInput").ap()
    # weights pre-duplicated on host into both partition halves
    w_ap = nc.dram_tensor("wt", [2 * C, KS * KS * OC], bf16,
                          kind="ExternalInput").ap()
    # chunk-major output: [img, chunk, oc, 3*128]; host untangles
    y_ap = nc.dram_tensor("yout", [IMGS, NCHP, OC, CHS], f16,
                          kind="ExternalOutput").ap()

    x_bc = x_ap.rearrange("b c l -> (b c) l")  # [128, XLEN]

    with tile.TileContext(nc) as tc:
        with tc.tile_pool(name="big", bufs=1) as big, \
             tc.tile_pool(name="psum", bufs=8, space="PSUM") as pp:

            x_sb = big.tile([128, XLEN], bf16, tag="xbuf")
            stage = big.tile([128, STLEN], f16, tag="stage")
            w_sb = big.tile([128, KS * KS * OC], bf16, tag="wbuf")

            # Weights head the sync ring (the scalar ring's first packet
            # lands ~0.9us later); x row segments alternate rings in strictly
            # increasing row order.  Both rings stripe across the same 16 DMA
            # engines (~365 GB/s aggregate), so rows land roughly in global
            # order ~1us+ ahead of the matmul stream's needs.
            nc.sync.dma_start(out=w_sb, in_=w_ap)
            segs = [(0, 4, nc.sync), (4, 8, nc.scalar), (8, 16, nc.sync),
                    (16, 28, nc.scalar), (28, 44, nc.sync),
                    (44, 60, nc.scalar), (60, 76, nc.sync),
                    (76, 92, nc.scalar), (92, 108, nc.sync),
                    (108, 124, nc.scalar), (124, HP, nc.sync)]
            for (a, b_, eng) in segs:
                eng.dma_start(out=x_sb[:, a * WP:b_ * WP],
                              in_=x_bc[:, a * WP:b_ * WP])

            def lhsT(half, t):
                return w_sb[half * 64:(half + 1) * 64, t * OC:(t + 1) * OC]

            def rhs(half, c, kh, kw, n):
                off = (RPC * c + kh) * WP + kw
                return x_sb[half * 64:(half + 1) * 64, off:off + n]

            TAPS = [(kh, kw) for kh in range(KS) for kw in range(KS)]

            def slot(i, p):
                return (i * NSLOT + p) * CHS

            # stage viewed as [part, slot, row, col] for the crop memsets
            st4 = stage.rearrange("p (s h w) -> p s h w", h=RPC, w=W)

            def evict(p, bank, i):
                """PSUM bank (img i, pair p) -> fp16 stage; crop zeroed by
                gpsimd memsets (pairs 7..13 touch output rows 44..83)."""
                src = bank[:, 0:CHN].rearrange(
                    "p (h w) -> p h w", w=WP)[:, :, 0:W]
                dst = stage[:, slot(i, p):slot(i, p) + CHS].rearrange(
                    "p (h w) -> p h w", w=W)
                if i == 0:
                    nc.vector.tensor_copy(dst, src)
                else:
                    nc.scalar.copy(dst, src)
                si = i * NSLOT + p
                if p == 7:            # chunk 14: row 44 only; chunk 15: all
                    nc.gpsimd.memset(st4[0:64, si, 2, CROP0:CROP1], 0.0)
                    nc.gpsimd.memset(st4[64:128, si, :, CROP0:CROP1], 0.0)
                elif 8 <= p <= 13:    # chunks 16..27: rows 48-83 all masked
                    nc.gpsimd.memset(st4[:, si, :, CROP0:CROP1], 0.0)

            def store_batch(s0, np_):
                """np_ pair-slots starting at slot s0, one DMA per image.
                All stores issue from the sync engine: the scalar queue must
                stay pure evictions, or a store dma_start waiting on crop
                memsets convoys behind it and stalls PSUM bank recycling."""
                for i in (0, 1):
                    src = stage[:, slot(i, s0):slot(i, s0) + np_ * CHS]
                    dst = y_ap[i, 2 * s0:2 * s0 + 2 * np_, :, :].rearrange(
                        "(pr par) o f -> (par o) pr f", par=2)
                    nc.sync.dma_start(out=dst,
                                      in_=src.rearrange("p (pr f) -> p pr f",
                                                        f=CHS))

            # PE warm-up: full-width (128x128) dummy matmuls on scratch SBUF
            # (stage slot written only much later) during the initial x-load
            # wait.  The HAM clock gate grants full speed one ~3.4us window
            # after sustained high-utilization PE activity begins, so the
            # dummies bridge from preamble end until the first rows land.
            dum = pp.tile([128, 512], f32, tag="ps", name="dum")
            scr = stage[:, slot(1, 20):slot(1, 20) + 512]
            for _ in range(N_DUMMY):
                nc.tensor.matmul(dum[:, 0:128], scr[:, 0:128],
                                 scr[:, 0:128], start=True, stop=True,
                                 skip_group_check=True)

            def leftover_block():
                """chunk 42 (rows 126/127): computed mid-stream at full
                four-quadrant width (row-col quadrant = (img, out-row)); both
                rows' outputs land on psum cols 0:129 with partitions
                (row, oc), evicted as one 128-partition copy per image."""
                pc_ = pp.tile([128, 512], f32, tag="ps", name="pc_")
                pd_ = pp.tile([128, 512], f32, tag="ps", name="pd_")
                for t, (kh, kw) in enumerate(TAPS):
                    st, sp = (t == 0), (t == len(TAPS) - 1)
                    for i, bank in ((0, pc_), (1, pd_)):
                        for r in (0, 1):   # out row 126 + r
                            off = (RPC * (NCH - 1) + r + kh) * WP + kw
                            nc.tensor.matmul(
                                bank[r * 64:(r + 1) * 64, 0:WP],
                                lhsT(i, t),
                                x_sb[i * 64:(i + 1) * 64, off:off + WP],
                                start=st, stop=sp, skip_group_check=True)
                for i, bank in ((0, pc_), (1, pd_)):
                    dst = stage[:, slot(i, 21):slot(i, 21) + W]
                    if i == 0:
                        nc.vector.tensor_copy(dst, bank[:, 0:W])
                    else:
                        nc.scalar.copy(dst, bank[:, 0:W])

            def store_leftover():
                """slot 21 partitions (row, oc) -> y chunks 42/43 col 0:W:
                row 126 to chunk 42, row 127 to the spare chunk 43 (the host
                reads row 127 from there)."""
                for i in (0, 1):
                    src = stage[:, slot(i, 21):slot(i, 21) + W]
                    dst = y_ap[i, NCH - 1:NCH + 1, :, 0:W].rearrange(
                        "n o w -> (n o) w")
                    nc.sync.dma_start(out=dst, in_=src)

            for p in range(NPAIR):
                ba = pp.tile([128, 512], f32, tag="ps", name=f"pa{p}")
                bb = pp.tile([128, 512], f32, tag="ps", name=f"pb{p}")
                for t, (kh, kw) in enumerate(TAPS):
                    st, sp = (t == 0), (t == len(TAPS) - 1)
                    for half, bank in ((0, ba), (1, bb)):
                        for c_par in (0, 1):
                            nc.tensor.matmul(
                                bank[c_par * 64:(c_par + 1) * 64, 0:CHN],
                                lhsT(half, t),
                                rhs(half, 2 * p + c_par, kh, kw, CHN),
                                start=st, stop=sp, skip_group_check=True)
                evict(p, ba, 0)
                evict(p, bb, 1)
                if p == 3:
                    store_batch(0, 4)
                elif p == 7:
                    store_batch(4, 4)
                elif p == 11:
                    store_batch(8, 4)
                elif p == 15:
                    store_batch(12, 4)
                elif p == 18:
                    store_batch(16, 3)
                elif p == 19:
                    store_batch(19, 1)
                elif p == 20:
                    store_batch(20, 1)

            # leftover chunk (2 output rows, 32KB/img store) goes last so the
            # post-matmul tail is as small as possible
            leftover_block()
            store_leftover()

    nc.compile()
    return nc


def _get_module():
    if "nc" not in _CACHE:
        _CACHE["nc"] = _build_module()
    return _CACHE["nc"]


def _make_in_maps(x, weight):
    x = np.asarray(x, dtype=np.float32)
    weight = np.asarray(weight, dtype=np.float32)
    # host marshaling: pad x into the row-major stride-129 layout
    xp = np.zeros((B, C, HP, WP), dtype=np.float32)
    xp[:, :, 1:H + 1, 1:W + 1] = x
    xp = xp.reshape(B, C, XLEN)
    import ml_dtypes
    xp = xp.astype(ml_dtypes.bfloat16)
    # weight [oc, ic, kh, kw] -> [ic, (kh kw), oc], duplicated in both halves
    wt = np.ascontiguousarray(
        weight.transpose(1, 2, 3, 0).reshape(C, KS * KS * OC)
    ).astype(ml_dtypes.bfloat16)
    wt = np.concatenate([wt, wt], axis=0)  # [128, 576]
    return [
        {"xin": np.ascontiguousarray(xp[k * IMGS:(k + 1) * IMGS]), "wt": wt}
        for k in range(N_CORES)
    ]


def kernel(x, weight):
    from concourse.bass_utils import run_bass_kernel_spmd

    nc = _get_module()
    in_maps = _make_in_maps(x, weight)
    res = run_bass_kernel_spmd(nc, in_maps, list(range(N_CORES)))
    # host unshard: [2, 44, 64, 384] fp16 chunk-major -> [2, 64, 128, 128]
    outs = []
    for k in range(N_CORES):
        y = np.asarray(res.results[k]["yout"])  # [IMGS, NCHP, OC, CHS] fp16
        y = y.reshape(IMGS, NCHP, OC, RPC, W).transpose(0, 2, 1, 3, 4)
        y = y.reshape(IMGS, OC, NCHP * RPC, W)
        y[:, :, H - 1, :] = y[:, :, RPC * NCH, :]  # row 127: chunk 43 col 0
        y = y[:, :, :H, :]
        outs.append(y.astype(np.float32))
    return np.concatenate(outs, axis=0)


# revision 14
# speedup vs baseline: 1.0646x; 1.0646x over previous
"""Trainium2 Bass kernel for CropConv: 3x3 same-padding conv (64->64 ch) on
[16, 64, 128, 128] fp32 input, with a static crop mask zeroing output rows/cols
[44:84).

Strategy (data-parallel over batch, 8 cores x 2 images each):
  - Host marshals x into a zero-padded row-major layout with row stride 129
    (131 padded rows; the left zero column of each row doubles as the previous
    row's right pad), so every conv tap (kh, kw) of an output row-chunk is one
    contiguous rhs slice.
  - Per core, image 0 lives in SBUF partitions 0-63 (partition = in-channel),
    image 1 in partitions 64-127.
  - The conv is 9 PSUM-accumulated TensorE matmuls per output chunk:
    out[oc, pix] += W[kh,kw][ic, oc].T @ x[ic, shifted pix].  K = M = 64, so
    four matmuls run concurrently in the four 64x64 quadrants of the PE array
    (row-half = image, col-half = chunk parity within the pair (2p, 2p+1)).
  - One pair (2 chunks x 2 images) per PSUM-bank group: finest-grained x-row
    dependency so the matmul stream starts as soon as the first 8 padded rows
    land and never starves (a starved >1us gap costs double: the HAM clock
    gate re-throttles the PE to 50% for its next ~3.4us window).
  - x input segments are interleaved across the sync and scalar HWDGE rings
    in strictly increasing row order so rows always arrive ahead of compute.
  - The crop mask is applied with cheap GPSIMD memsets on the staged output
    (no mask tensor, no mask DMA) and evictions are uniform: PSUM -> fp16
    stage via Vector (img 0) and Scalar-ACT (img 1) copies.
  - Stores are batched 4 slots early on; the leftover chunk 42 (rows 126/127,
    computed mid-stream at full four-quadrant width) and the last two pairs
    are stored individually so the post-matmul tail is short.
  - PE warm-up: full-width dummy matmuls during the initial x-load wait keep
    the HAM util window high so full clock is granted by the time real data
    lands.  The host untangles the chunk-major fp16 output and upcasts.
"""

import numpy as np

# ---- problem constants (hardcoded; kernel.py must be self-contained) ----
B, C, H, W = 16, 64, 128, 128
OC, KS = 64, 3
N_CORES = 8
IMGS = B // N_CORES  # 2 images per core

WP = W + 1            # padded row stride: 129
HP = H + 3            # padded rows in the x buffer: 131
XLEN = HP * WP        # 16899 elems per partition

RPC = 3               # output rows per chunk
NCH = (H + RPC - 1) // RPC   # 43 chunks per image (last has 2 rows)
NCHP = NCH + 1        # padded to 44 (chunk 43 is never written, dropped)
NPAIR = 21            # adjacent-chunk pairs (2p, 2p+1); chunk 42 leftover
NSLOT = 22            # stage slots per image: 21 pairs + leftover
CHN = RPC * WP        # matmul free dim per full chunk: 387
CHS = RPC * W         # compact stage slot stride: 384
STLEN = 2 * NSLOT * CHS   # stage free size: 16896

CROP0, CROP1 = 44, 84  # masked rows/cols [44, 84)
# PE warm-up matmuls: must bridge the whole preamble-end .. first-x-rows-land
# window with NO gap, or the HAM utilization window (~3.4us granularity,
# ~75% threshold) misses the grant and the PE runs at 50% for extra windows.
N_DUMMY = 32

_CACHE = {}


def _build_module():
    import concourse.tile as tile
    from concourse import bacc, mybir

    f32 = mybir.dt.float32
    f16 = mybir.dt.float16
    bf16 = mybir.dt.bfloat16

    nc = bacc.Bacc("TRN2", target_bir_lowering=False, debug=False,
                   num_devices=N_CORES)

    x_ap = nc.dram_tensor("xin", [IMGS, C, XLEN], bf16,
                          kind="ExternalInput").ap()
    # weights pre-duplicated on host into both partition halves
    w_ap = nc.dram_tensor("wt", [2 * C, KS * KS * OC], bf16,
                          kind="ExternalInput").ap()
    # chunk-major output: [img, chunk, oc, 3*128]; host untangles
    y_ap = nc.dram_tensor("yout", [IMGS, NCHP, OC, CHS], f16,
                          kind="ExternalOutput").ap()

    x_bc = x_ap.rearrange("b c l -> (b c) l")  # [128, XLEN]

    with tile.TileContext(nc) as tc:
        with tc.tile_pool(name="big", bufs=1) as big, \
             tc.tile_pool(name="psum", bufs=8, space="PSUM") as pp:

            x_sb = big.tile([128, XLEN], bf16, tag="xbuf")
            stage = big.tile([128, STLEN], f16, tag="stage")
            w_sb = big.tile([128, KS * KS * OC], bf16, tag="wbuf")

            # Weights head the scalar ring, the first x rows the sync ring,
            # so the head of the critical path is split across both rings
            # (early small-element DMAs only move ~50-130 KB/us per ring);
            # later x segments alternate rings in strictly increasing row
            # order so rows always land ~1us+ ahead of the matmul stream.
            nc.scalar.dma_start(out=w_sb, in_=w_ap)
            segs = [(0, 8, nc.sync), (8, 16, nc.scalar), (16, 28, nc.sync),
                    (28, 44, nc.scalar), (44, 60, nc.sync),
                    (60, 76, nc.scalar), (76, 92, nc.sync),
                    (92, 108, nc.scalar), (108, 124, nc.sync),
                    (124, HP, nc.scalar)]
            for (a, b_, eng) in segs:
                eng.dma_start(out=x_sb[:, a * WP:b_ * WP],
                              in_=x_bc[:, a * WP:b_ * WP])

            def lhsT(half, t):
                return w_sb[half * 64:(half + 1) * 64, t * OC:(t + 1) * OC]

            def rhs(half, c, kh, kw, n):
                off = (RPC * c + kh) * WP + kw
                return x_sb[half * 64:(half + 1) * 64, off:off + n]

            TAPS = [(kh, kw) for kh in range(KS) for kw in range(KS)]

            def slot(i, p):
                return (i * NSLOT + p) * CHS

            # stage viewed as [part, slot, row, col] for the crop memsets
            st4 = stage.rearrange("p (s h w) -> p s h w", h=RPC, w=W)

            def evict(p, bank, i):
                """PSUM bank (img i, pair p) -> fp16 stage; crop zeroed by
                gpsimd memsets (pairs 7..13 touch output rows 44..83)."""
                src = bank[:, 0:CHN].rearrange(
                    "p (h w) -> p h w", w=WP)[:, :, 0:W]
                dst = stage[:, slot(i, p):slot(i, p) + CHS].rearrange(
                    "p (h w) -> p h w", w=W)
                if i == 0:
                    nc.vector.tensor_copy(dst, src)
                else:
                    nc.scalar.copy(dst, src)
                si = i * NSLOT + p
                if p == 7:            # chunk 14: row 44 only; chunk 15: all
                    nc.gpsimd.memset(st4[0:64, si, 2, CROP0:CROP1], 0.0)
                    nc.gpsimd.memset(st4[64:128, si, :, CROP0:CROP1], 0.0)
                elif 8 <= p <= 13:    # chunks 16..27: rows 48-83 all masked
                    nc.gpsimd.memset(st4[:, si, :, CROP0:CROP1], 0.0)

            def store_batch(s0, np_):
                """np_ pair-slots starting at slot s0, one DMA per image,
                split across both rings (a single ring only sustains
                ~180 KB/us; all-stores-on-one-ring trails the kernel end).
                Each batch is issued >=1 pair after its last eviction so its
                crop-memset semaphores are already satisfied -- a waiting
                store dma_start at the scalar queue head convoys the next
                eviction and stalls PSUM bank recycling."""
                for i, eng in ((0, nc.sync), (1, nc.scalar)):
                    src = stage[:, slot(i, s0):slot(i, s0) + np_ * CHS]
                    dst = y_ap[i, 2 * s0:2 * s0 + 2 * np_, :, :].rearrange(
                        "(pr par) o f -> (par o) pr f", par=2)
                    eng.dma_start(out=dst,
                                  in_=src.rearrange("p (pr f) -> p pr f",
                                                    f=CHS))

            # PE warm-up: full-width (128x128) dummy matmuls on scratch SBUF
            # (stage slot written only much later) during the initial x-load
            # wait.  The HAM clock gate grants full speed one ~3.4us window
            # after sustained high-utilization PE activity begins, so the
            # dummies bridge from preamble end until the first rows land.
            dum = pp.tile([128, 512], f32, tag="ps", name="dum")
            scr = stage[:, slot(1, 20):slot(1, 20) + 512]
            for _ in range(N_DUMMY):
                nc.tensor.matmul(dum[:, 0:128], scr[:, 0:128],
                                 scr[:, 0:128], start=True, stop=True,
                                 skip_group_check=True)

            def leftover_block():
                """chunk 42 (rows 126/127): computed mid-stream at full
                four-quadrant width (row-col quadrant = (img, out-row)); both
                rows' outputs land on psum cols 0:129 with partitions
                (row, oc), evicted as one 128-partition copy per image."""
                pc_ = pp.tile([128, 512], f32, tag="ps", name="pc_")
                pd_ = pp.tile([128, 512], f32, tag="ps", name="pd_")
                for t, (kh, kw) in enumerate(TAPS):
                    st, sp = (t == 0), (t == len(TAPS) - 1)
                    for i, bank in ((0, pc_), (1, pd_)):
                        for r in (0, 1):   # out row 126 + r
                            off = (RPC * (NCH - 1) + r + kh) * WP + kw
                            nc.tensor.matmul(
                                bank[r * 64:(r + 1) * 64, 0:WP],
                                lhsT(i, t),
                                x_sb[i * 64:(i + 1) * 64, off:off + WP],
                                start=st, stop=sp, skip_group_check=True)
                for i, bank in ((0, pc_), (1, pd_)):
                    dst = stage[:, slot(i, 21):slot(i, 21) + W]
                    if i == 0:
                        nc.vector.tensor_copy(dst, bank[:, 0:W])
                    else:
                        nc.scalar.copy(dst, bank[:, 0:W])

            def store_leftover():
                """slot 21 partitions (row, oc) -> y chunks 42/43 col 0:W:
                row 126 to chunk 42, row 127 to the spare chunk 43 (the host
                reads row 127 from there)."""
                for i in (0, 1):
                    src = stage[:, slot(i, 21):slot(i, 21) + W]
                    dst = y_ap[i, NCH - 1:NCH + 1, :, 0:W].rearrange(
                        "n o w -> (n o) w")
                    nc.sync.dma_start(out=dst, in_=src)

            for p in range(NPAIR):
                ba = pp.tile([128, 512], f32, tag="ps", name=f"pa{p}")
                bb = pp.tile([128, 512], f32, tag="ps", name=f"pb{p}")
                for t, (kh, kw) in enumerate(TAPS):
                    st, sp = (t == 0), (t == len(TAPS) - 1)
                    for half, bank in ((0, ba), (1, bb)):
                        for c_par in (0, 1):
                            nc.tensor.matmul(
                                bank[c_par * 64:(c_par + 1) * 64, 0:CHN],
                                lhsT(half, t),
                                rhs(half, 2 * p + c_par, kh, kw, CHN),
                                start=st, stop=sp, skip_group_check=True)
                evict(p, ba, 0)
                evict(p, bb, 1)
                if p == 3:
                    store_batch(0, 4)
                elif p == 9:       # slots 4-7: +2 pairs past pair-7 memsets
                    store_batch(4, 4)
                elif p == 12:      # slots 8-11: +1 pair past pair-11 memsets
                    store_batch(8, 4)
                elif p == 16:      # slots 12-15: pair-13 memsets long done
                    store_batch(12, 4)
                elif p == 18:
                    store_batch(16, 3)
                elif p == 19:
                    store_batch(19, 1)
                elif p == 20:
                    store_batch(20, 1)

            # leftover chunk (2 output rows, 32KB/img store) goes last so the
            # post-matmul tail is as small as possible
            leftover_block()
            store_leftover()

    nc.compile()
    return nc


def _get_module():
    if "nc" not in _CACHE:
        _CACHE["nc"] = _build_module()
    return _CACHE["nc"]


def _make_in_maps(x, weight):
    x = np.asarray(x, dtype=np.float32)
    weight = np.asarray(weight, dtype=np.float32)
    # host marshaling: pad x into the row-major stride-129 layout
    xp = np.zeros((B, C, HP, WP), dtype=np.float32)
    xp[:, :, 1:H + 1, 1:W + 1] = x
    xp = xp.reshape(B, C, XLEN)
    import ml_dtypes
    xp = xp.astype(ml_dtypes.bfloat16)
    # weight [oc, ic, kh, kw] -> [ic, (kh kw), oc], duplicated in both halves
    wt = np.ascontiguousarray(
        weight.transpose(1, 2, 3, 0).reshape(C, KS * KS * OC)
    ).astype(ml_dtypes.bfloat16)
    wt = np.concatenate([wt, wt], axis=0)  # [128, 576]
    return [
        {"xin": np.ascontiguousarray(xp[k * IMGS:(k + 1) * IMGS]), "wt": wt}
        for k in range(N_CORES)
    ]


def kernel(x, weight):
    from concourse.bass_utils import run_bass_kernel_spmd

    nc = _get_module()
    in_maps = _make_in_maps(x, weight)
    res = run_bass_kernel_spmd(nc, in_maps, list(range(N_CORES)))
    # host unshard: [2, 44, 64, 384] fp16 chunk-major -> [2, 64, 128, 128]
    outs = []
    for k in range(N_CORES):
        y = np.asarray(res.results[k]["yout"])  # [IMGS, NCHP, OC, CHS] fp16
        y = y.reshape(IMGS, NCHP, OC, RPC, W).transpose(0, 2, 1, 3, 4)
        y = y.reshape(IMGS, OC, NCHP * RPC, W)
        y[:, :, H - 1, :] = y[:, :, RPC * NCH, :]  # row 127: chunk 43 col 0
        y = y[:, :, :H, :]
        outs.append(y.astype(np.float32))
    return np.concatenate(outs, axis=0)


# revision 20
# speedup vs baseline: 1.1024x; 1.0355x over previous
"""Trainium2 Bass kernel for CropConv: 3x3 same-padding conv (64->64 ch) on
[16, 64, 128, 128] fp32 input, with a static crop mask zeroing output rows/cols
[44:84).

Strategy (data-parallel over batch, 8 cores x 2 images each):
  - Host marshals x into a zero-padded row-major layout with row stride 129
    (131 padded rows; the left zero column of each row doubles as the previous
    row's right pad), so every conv tap (kh, kw) of an output row-chunk is one
    contiguous rhs slice.
  - Per core, image 0 lives in SBUF partitions 0-63 (partition = in-channel),
    image 1 in partitions 64-127.
  - The conv is 9 PSUM-accumulated TensorE matmuls per output chunk:
    out[oc, pix] += W[kh,kw][ic, oc].T @ x[ic, shifted pix].  K = M = 64, so
    four matmuls run concurrently in the four 64x64 quadrants of the PE array
    (row-half = image, col-half = chunk parity within the pair (2p, 2p+1)).
  - One pair (2 chunks x 2 images) per PSUM-bank group: finest-grained x-row
    dependency so the matmul stream starts as soon as the first 8 padded rows
    land and never starves (a starved >1us gap costs double: the HAM clock
    gate re-throttles the PE to 50% for its next ~3.4us window).
  - x input segments are interleaved across the sync and scalar HWDGE rings
    in strictly increasing row order so rows always arrive ahead of compute.
  - The crop mask is applied with cheap GPSIMD memsets on the staged output
    (no mask tensor, no mask DMA) and evictions are uniform: PSUM -> fp16
    stage via Vector (img 0) and Scalar-ACT (img 1) copies.
  - Stores are batched 4 slots early on; the leftover chunk 42 (rows 126/127,
    computed mid-stream at full four-quadrant width) and the last two pairs
    are stored individually so the post-matmul tail is short.
  - PE warm-up: full-width dummy matmuls during the initial x-load wait keep
    the HAM util window high so full clock is granted by the time real data
    lands.  The host untangles the chunk-major fp16 output and upcasts.
"""

import numpy as np

# ---- problem constants (hardcoded; kernel.py must be self-contained) ----
B, C, H, W = 16, 64, 128, 128
OC, KS = 64, 3
N_CORES = 8
IMGS = B // N_CORES  # 2 images per core

WP = W + 1            # padded row stride: 129
HP = H + 3            # padded rows in the x buffer: 131
XLEN = HP * WP        # 16899 elems per partition

RPC = 3               # output rows per chunk
NCH = (H + RPC - 1) // RPC   # 43 chunks per image (last has 2 rows)
NCHP = NCH + 1        # padded to 44 (chunk 43 is never written, dropped)
NPAIR = 21            # adjacent-chunk pairs (2p, 2p+1); chunk 42 leftover
NSLOT = 22            # stage slots per image: 21 pairs + leftover
CHN = RPC * WP        # matmul free dim per full chunk: 387
CHS = RPC * W         # compact stage slot stride: 384
STLEN = 2 * NSLOT * CHS   # stage free size: 16896

CROP0, CROP1 = 44, 84  # masked rows/cols [44, 84)
# PE warm-up matmuls: must bridge the whole preamble-end .. first-x-rows-land
# window with NO gap, or the HAM utilization window (~3.4us granularity,
# ~75% threshold) misses the grant and the PE runs at 50% for extra windows.
N_DUMMY = 32

_CACHE = {}


def _build_module():
    import concourse.tile as tile
    from concourse import bacc, mybir
    from concourse.ap import AP as _AP

    f32 = mybir.dt.float32
    f16 = mybir.dt.float16
    bf16 = mybir.dt.bfloat16

    nc = bacc.Bacc("TRN2", target_bir_lowering=False, debug=False,
                   num_devices=N_CORES)

    x_ap = nc.dram_tensor("xin", [IMGS, C, XLEN], bf16,
                          kind="ExternalInput").ap()
    # weights pre-duplicated on host into both partition halves
    w_ap = nc.dram_tensor("wt", [2 * C, KS * KS * OC], bf16,
                          kind="ExternalInput").ap()
    # chunk-major output: [img, chunk, oc, 3*128]; host untangles
    y_ap = nc.dram_tensor("yout", [IMGS, NCHP, OC, CHS], f16,
                          kind="ExternalOutput").ap()

    x_bc = x_ap.rearrange("b c l -> (b c) l")  # [128, XLEN]

    with tile.TileContext(nc) as tc:
        with tc.tile_pool(name="big", bufs=1) as big, \
             tc.tile_pool(name="psum", bufs=8, space="PSUM") as pp:

            x_sb = big.tile([128, XLEN], bf16, tag="xbuf")
            stage = big.tile([128, STLEN], f16, tag="stage")
            w_sb = big.tile([128, KS * KS * OC], bf16, tag="wbuf")

            # Weights head the scalar ring, the first x rows the sync ring,
            # so the head of the critical path is split across both rings
            # (early small-element DMAs only move ~50-130 KB/us per ring);
            # later x segments alternate rings in strictly increasing row
            # order so rows always land ~1us+ ahead of the matmul stream.
            nc.scalar.dma_start(out=w_sb, in_=w_ap)
            segs = [(0, 8, nc.sync), (8, 16, nc.scalar), (16, 28, nc.sync),
                    (28, 44, nc.scalar), (44, 60, nc.sync),
                    (60, 76, nc.scalar), (76, 92, nc.sync),
                    (92, 108, nc.scalar), (108, 124, nc.sync),
                    (124, HP, nc.scalar)]
            for (a, b_, eng) in segs:
                eng.dma_start(out=x_sb[:, a * WP:b_ * WP],
                              in_=x_bc[:, a * WP:b_ * WP])

            def lhsT(half, t):
                return w_sb[half * 64:(half + 1) * 64, t * OC:(t + 1) * OC]

            def rhs(half, c, kh, kw, n):
                off = (RPC * c + kh) * WP + kw
                return x_sb[half * 64:(half + 1) * 64, off:off + n]

            TAPS = [(kh, kw) for kh in range(KS) for kw in range(KS)]

            def slot(i, p):
                return (i * NSLOT + p) * CHS

            # stage viewed as [part, slot, row, col] for the crop memsets
            st4 = stage.rearrange("p (s h w) -> p s h w", h=RPC, w=W)

            KCW = CROP0  # kept cols per side: [0,44) and [84,128)

            def ap4(base, row_stride):
                """[part, row, side, keep-col] view of a 3-row chunk span:
                side 0 = cols [0,44), side 1 = cols [84,128)."""
                pat = [list(q) for q in base.ap]
                return _AP(base.tensor, base.offset,
                           [pat[0], [row_stride, RPC], [CROP1, 2], [1, KCW]])

            def masked_full(c):
                """chunks whose 3 output rows all lie in the crop rows."""
                return 15 <= c <= 27

            def evict(p, bank, i):
                """PSUM bank (img i, pair p) -> fp16 stage; crop zeroed by
                gpsimd memsets (pairs 7..13 touch output rows 44..83).
                Fully-masked chunks only computed/evicted outside the crop
                cols, so their psum crop cols (stale data) are never read."""
                c0m, c1m = masked_full(2 * p), masked_full(2 * p + 1)
                so = slot(i, p)
                copy = nc.vector.tensor_copy if i == 0 else nc.scalar.copy
                if c0m and c1m:
                    copy(ap4(stage[:, so:so + CHS], W),
                         ap4(bank[:, 0:CHN], WP))
                elif c1m:      # pair 7: chunk 14 full, chunk 15 piece-wise
                    copy(stage[0:64, so:so + CHS].rearrange(
                             "p (h w) -> p h w", w=W),
                         bank[0:64, 0:CHN].rearrange(
                             "p (h w) -> p h w", w=WP)[:, :, 0:W])
                    copy(ap4(stage[64:128, so:so + CHS], W),
                         ap4(bank[64:128, 0:CHN], WP))
                else:
                    copy(stage[:, so:so + CHS].rearrange(
                             "p (h w) -> p h w", w=W),
                         bank[:, 0:CHN].rearrange(
                             "p (h w) -> p h w", w=WP)[:, :, 0:W])
                si = i * NSLOT + p
                if p == 7:            # chunk 14: row 44 only; chunk 15: all
                    nc.gpsimd.memset(st4[0:64, si, 2, CROP0:CROP1], 0.0)
                    nc.gpsimd.memset(st4[64:128, si, :, CROP0:CROP1], 0.0)
                elif 8 <= p <= 13:    # chunks 16..27: rows 48-83 all masked
                    nc.gpsimd.memset(st4[:, si, :, CROP0:CROP1], 0.0)

            def store_batch(s0, np_):
                """np_ pair-slots starting at slot s0, one DMA per image,
                split across both rings (a single ring only sustains
                ~180 KB/us; all-stores-on-one-ring trails the kernel end).
                Each batch is issued >=1 pair after its last eviction so its
                crop-memset semaphores are already satisfied -- a waiting
                store dma_start at the scalar queue head convoys the next
                eviction and stalls PSUM bank recycling."""
                for i, eng in ((0, nc.sync), (1, nc.scalar)):
                    src = stage[:, slot(i, s0):slot(i, s0) + np_ * CHS]
                    dst = y_ap[i, 2 * s0:2 * s0 + 2 * np_, :, :].rearrange(
                        "(pr par) o f -> (par o) pr f", par=2)
                    eng.dma_start(out=dst,
                                  in_=src.rearrange("p (pr f) -> p pr f",
                                                    f=CHS))

            # PE warm-up: full-width (128x128) dummy matmuls on scratch SBUF
            # (stage slot written only much later) during the initial x-load
            # wait.  The HAM clock gate grants full speed one ~3.4us window
            # after sustained high-utilization PE activity begins, so the
            # dummies bridge from preamble end until the first rows land.
            dum = pp.tile([128, 512], f32, tag="ps", name="dum")
            scr = stage[:, slot(1, 20):slot(1, 20) + 512]
            for _ in range(N_DUMMY):
                nc.tensor.matmul(dum[:, 0:128], scr[:, 0:128],
                                 scr[:, 0:128], start=True, stop=True,
                                 skip_group_check=True)

            def leftover_block():
                """chunk 42 (rows 126/127): computed mid-stream at full
                four-quadrant width (row-col quadrant = (img, out-row)); both
                rows' outputs land on psum cols 0:129 with partitions
                (row, oc), evicted as one 128-partition copy per image."""
                pc_ = pp.tile([128, 512], f32, tag="ps", name="pc_")
                pd_ = pp.tile([128, 512], f32, tag="ps", name="pd_")
                for t, (kh, kw) in enumerate(TAPS):
                    st, sp = (t == 0), (t == len(TAPS) - 1)
                    for i, bank in ((0, pc_), (1, pd_)):
                        for r in (0, 1):   # out row 126 + r
                            off = (RPC * (NCH - 1) + r + kh) * WP + kw
                            nc.tensor.matmul(
                                bank[r * 64:(r + 1) * 64, 0:WP],
                                lhsT(i, t),
                                x_sb[i * 64:(i + 1) * 64, off:off + WP],
                                start=st, stop=sp, skip_group_check=True)
                for i, bank in ((0, pc_), (1, pd_)):
                    dst = stage[:, slot(i, 21):slot(i, 21) + W]
                    if i == 0:
                        nc.vector.tensor_copy(dst, bank[:, 0:W])
                    else:
                        nc.scalar.copy(dst, bank[:, 0:W])

            def store_leftover():
                """slot 21 partitions (row, oc) -> y chunks 42/43 col 0:W:
                row 126 to chunk 42, row 127 to the spare chunk 43 (the host
                reads row 127 from there)."""
                for i, eng in ((0, nc.sync), (1, nc.scalar)):
                    src = stage[:, slot(i, 21):slot(i, 21) + W]
                    dst = y_ap[i, NCH - 1:NCH + 1, :, 0:W].rearrange(
                        "n o w -> (n o) w")
                    eng.dma_start(out=dst, in_=src)

            for p in range(NPAIR):
                if p == 20:
                    # leftover computed just before the last pair so its tiny
                    # store (32KB/img) overlaps pair 20's matmuls and the
                    # post-matmul tail stays minimal
                    leftover_block()
                    store_leftover()
                ba = pp.tile([128, 512], f32, tag="ps", name=f"pa{p}")
                bb = pp.tile([128, 512], f32, tag="ps", name=f"pb{p}")
                for t, (kh, kw) in enumerate(TAPS):
                    st, sp = (t == 0), (t == len(TAPS) - 1)
                    for half, bank in ((0, ba), (1, bb)):
                        for c_par in (0, 1):
                            c = 2 * p + c_par
                            o_ = bank[c_par * 64:(c_par + 1) * 64, 0:CHN]
                            r_ = rhs(half, c, kh, kw, CHN)
                            if masked_full(c):   # skip the 40 crop cols
                                o_, r_ = ap4(o_, WP), ap4(r_, WP)
                            nc.tensor.matmul(o_, lhsT(half, t), r_,
                                             start=st, stop=sp,
                                             skip_group_check=True)
                evict(p, ba, 0)
                evict(p, bb, 1)
                if p == 3:
                    store_batch(0, 4)
                elif p == 9:       # slots 4-7: +2 pairs past pair-7 memsets
                    store_batch(4, 4)
                elif p == 12:      # slots 8-11: +1 pair past pair-11 memsets
                    store_batch(8, 4)
                elif p == 16:      # slots 12-15: pair-13 memsets long done
                    store_batch(12, 4)
                elif p == 18:
                    store_batch(16, 3)
                elif p == 19:
                    store_batch(19, 1)
                elif p == 20:
                    store_batch(20, 1)

    nc.compile()
    return nc


def _get_module():
    if "nc" not in _CACHE:
        _CACHE["nc"] = _build_module()
    return _CACHE["nc"]


def _make_in_maps(x, weight):
    x = np.asarray(x, dtype=np.float32)
    weight = np.asarray(weight, dtype=np.float32)
    # host marshaling: pad x into the row-major stride-129 layout
    xp = np.zeros((B, C, HP, WP), dtype=np.float32)
    xp[:, :, 1:H + 1, 1:W + 1] = x
    xp = xp.reshape(B, C, XLEN)
    import ml_dtypes
    xp = xp.astype(ml_dtypes.bfloat16)
    # weight [oc, ic, kh, kw] -> [ic, (kh kw), oc], duplicated in both halves
    wt = np.ascontiguousarray(
        weight.transpose(1, 2, 3, 0).reshape(C, KS * KS * OC)
    ).astype(ml_dtypes.bfloat16)
    wt = np.concatenate([wt, wt], axis=0)  # [128, 576]
    return [
        {"xin": np.ascontiguousarray(xp[k * IMGS:(k + 1) * IMGS]), "wt": wt}
        for k in range(N_CORES)
    ]


def kernel(x, weight):
    from concourse.bass_utils import run_bass_kernel_spmd

    nc = _get_module()
    in_maps = _make_in_maps(x, weight)
    res = run_bass_kernel_spmd(nc, in_maps, list(range(N_CORES)))
    # host unshard: [2, 44, 64, 384] fp16 chunk-major -> [2, 64, 128, 128]
    outs = []
    for k in range(N_CORES):
        y = np.asarray(res.results[k]["yout"])  # [IMGS, NCHP, OC, CHS] fp16
        y = y.reshape(IMGS, NCHP, OC, RPC, W).transpose(0, 2, 1, 3, 4)
        y = y.reshape(IMGS, OC, NCHP * RPC, W)
        y[:, :, H - 1, :] = y[:, :, RPC * NCH, :]  # row 127: chunk 43 col 0
        y = y[:, :, :H, :]
        outs.append(y.astype(np.float32))
    return np.concatenate(outs, axis=0)
